# revision 1
# baseline (speedup 1.0000x reference)
"""Trainium2 Bass kernel for DUPN-style LSTM + windowed-softmax attention pooling.

Math (per batch element b):
  LSTM over T=128 steps (torch gate order), hidden H=512, input D=256.
  a[t] = sigmoid(x[t]·u1 + h[t]·u2), u1 = (v1@A1)^T, u2 = (v1@A2)^T  (folded)
  out[b,k,:] = softmax-pooled sum of h[t] over window t <= t_k, for 4 slots.

Sharding: data-parallel over batch, 32 per core x 8 cores, weights replicated.

Per-core device schedule:
  - xw = x@W_ih^T + bias precomputed in row-chunks of 128 rows (4 timesteps),
    fused into the loop as a prefetch, kept in an SBUF ring. Bias folded in
    via a K=1 ones-row matmul. Computed in two [128, 1024] halves to fit PSUM.
  - Per step: z [32, 2048] accumulated in a 4-bank PSUM tile: 4 identity
    matmuls inject xw rows (K=32), then 16 k-pass matmuls add h @ W_hh^T.
    All matmul outputs start at PSUM partition 0 (walrus emits col_grp=0xf
    only; non-zero dst partitions are unencodable).
  - Matmul operands are fp32r (fp32 rounded to 11 mantissa bits; full PE rate
    at N>=256). walrus requires producer dtype = fp32r, so matmul-feeding
    tiles are declared fp32r and written by converting copies.
  - Gates are free-dim slices of z (order i,f,o,g): sigmoid on [:, 0:1536],
    tanh on [:, 1536:2048]; c/h updates on DVE, everything at partition 0.
  - h transposed to hT via 4 PE transposes into hsT_store (fp32r), which is
    the next step's matmul stationary and the pooling source.
  - Post-loop: a = sigmoid(a1+a2), windowed softmax with host-built masks,
    pooling via per-b [4,T]@[T,H] matmuls.
"""
import sys

if "/opt/trn_rl_repo" not in sys.path:
    sys.path.insert(0, "/opt/trn_rl_repo")

import numpy as np
import concourse.bass as bass
import concourse.bacc as bacc
import concourse.tile as tile
from concourse import mybir
from concourse.bass_utils import run_bass_kernel_spmd
from contextlib import ExitStack

F32 = mybir.dt.float32
F32R = mybir.dt.float32r
AFT = mybir.ActivationFunctionType
ALU = mybir.AluOpType

T, BF, D, H, K, NC = 128, 256, 256, 512, 4, 8
BL = BF // NC          # 32 batch per core
G = 4 * H              # 2048
NEG_INF = -1e9

_cached = {}


def _build_program(t_steps=T):
    nc = bacc.Bacc()
    # ---- DRAM I/O (fp32r where feeding matmuls; same bytes as fp32) ----
    d_xT = nc.declare_dram_parameter("xT", [D, t_steps * BL], F32R, isOutput=False)
    d_wih = nc.declare_dram_parameter("wih", [D, G], F32R, isOutput=False)
    d_whh = nc.declare_dram_parameter("whh", [H, G], F32R, isOutput=False)
    d_biasrow = nc.declare_dram_parameter("biasrow", [1, G], F32R, isOutput=False)
    d_ones = nc.declare_dram_parameter("onesrow", [1, 128], F32R, isOutput=False)
    d_u1t = nc.declare_dram_parameter("u1t", [128, 2 * (D // 128)], F32R, isOutput=False)
    d_u2b = nc.declare_dram_parameter("u2b", [BL, H], F32, isOutput=False)
    d_i32s = nc.declare_dram_parameter("i32s", [128, 32], F32, isOutput=False)
    d_i128 = nc.declare_dram_parameter("i128", [128, 128], F32, isOutput=False)
    d_maskneg = nc.declare_dram_parameter("maskneg", [BL, K * t_steps], F32, isOutput=False)
    d_valid = nc.declare_dram_parameter("valid", [BL, K], F32, isOutput=False)
    d_out = nc.declare_dram_parameter("out", [BL * K, H], F32, isOutput=True)

    NRC = t_steps // 4     # row chunks of 128 rows (4 timesteps each)

    with tile.TileContext(nc) as tc, ExitStack() as ctx:
        nv, ns, nt, ng = nc.vector, nc.scalar, nc.tensor, nc.gpsimd

        consts = ctx.enter_context(tc.tile_pool(name="consts", bufs=1))
        big = ctx.enter_context(tc.tile_pool(name="big", bufs=1))

        # ---- load constants ----
        wih_sb = [consts.tile([128, G], F32R, tag=f"wih{i}", name=f"wih{i}")
                  for i in range(2)]
        for i in range(2):
            nc.sync.dma_start(wih_sb[i][:], d_wih[128 * i:128 * (i + 1), :])
        whh_sb = [consts.tile([128, G], F32R, tag=f"whh{i}", name=f"whh{i}")
                  for i in range(4)]
        for i in range(4):
            nc.sync.dma_start(whh_sb[i][:], d_whh[128 * i:128 * (i + 1), :])
        biasrow_sb = consts.tile([1, G], F32R, tag="biasrow")
        nc.sync.dma_start(biasrow_sb[:], d_biasrow[:])
        ones_sb = consts.tile([1, 128], F32R, tag="ones")
        nc.sync.dma_start(ones_sb[:], d_ones[:])
        u1t_sb = consts.tile([128, 4], F32R, tag="u1t")
        nc.sync.dma_start(u1t_sb[:], d_u1t[:])
        u2b_sb = consts.tile([BL, H], F32, tag="u2b")
        nc.sync.dma_start(u2b_sb[:], d_u2b[:])
        i32s_r = consts.tile([128, 32], F32R, tag="i32s_r")
        nc.sync.dma_start(i32s_r[:], d_i32s[:].bitcast(F32R))
        i32s_f = consts.tile([128, 32], F32, tag="i32s_f")
        nc.sync.dma_start(i32s_f[:], d_i32s[:])
        i128_r = consts.tile([128, 128], F32R, tag="i128_r")
        nc.sync.dma_start(i128_r[:], d_i128[:].bitcast(F32R))
        maskneg_sb = consts.tile([BL, K * t_steps], F32, tag="maskneg")
        nc.sync.dma_start(maskneg_sb[:], d_maskneg[:])
        valid_sb = consts.tile([BL, K], F32, tag="valid")
        nc.sync.dma_start(valid_sb[:], d_valid[:])

        # ---- persistent state ----
        hsT = big.tile([128, t_steps * 128], F32R, tag="hsT")      # [p, t*128+c*32+b]
        c_sb = big.tile([BL, H], F32, tag="c")
        a1ch = big.tile([128, NRC], F32, tag="a1ch")               # a1 by row-chunk
        a2_sb = big.tile([BL, t_steps], F32, tag="a2")

        # ---- loop pools ----
        loop_ctx = ExitStack()
        xt_pool = loop_ctx.enter_context(tc.tile_pool(name="xt", bufs=2))
        xw_pool = loop_ctx.enter_context(tc.tile_pool(name="xw", bufs=2))
        gate_pool = loop_ctx.enter_context(tc.tile_pool(name="gate", bufs=2))
        tmp_pool = loop_ctx.enter_context(tc.tile_pool(name="tmp", bufs=2))
        h_pool = loop_ctx.enter_context(tc.tile_pool(name="h", bufs=2))
        scr_pool = loop_ctx.enter_context(tc.tile_pool(name="scr", bufs=1))
        ps_xw = loop_ctx.enter_context(tc.tile_pool(name="ps_xw", bufs=1, space="PSUM"))
        ps_z = loop_ctx.enter_context(tc.tile_pool(name="ps_z", bufs=1, space="PSUM"))
        ps_hT = loop_ctx.enter_context(tc.tile_pool(name="ps_hT", bufs=1, space="PSUM"))
        ps_a1 = loop_ctx.enter_context(tc.tile_pool(name="ps_a1", bufs=1, space="PSUM"))

        def emit_xw_chunk(r):
            """xw rows 128r..128r+128 (timesteps 4r..4r+3) -> xw ring + a1 col r.

            Two [128, 1024] PSUM halves (2 banks each, bufs=2) to stay in
            budget: ps_xw 2x2 + ps_z 4 + ps_hT 1 + ps_a1 1 = 8 banks.
            """
            xtc = [xt_pool.tile([128, 128], F32R, tag=f"xtc{kd}", name=f"xtc{kd}_{r}")
                   for kd in range(2)]
            for kd in range(2):
                nc.sync.dma_start(xtc[kd][:],
                                  d_xT[128 * kd:128 * (kd + 1), 128 * r:128 * (r + 1)])
            xw = xw_pool.tile([128, G], F32R, tag="xw")
            for half in range(2):
                pxw = ps_xw.tile([128, 1024], F32, tag="pxw")
                for kd in range(2):
                    for n in range(2):
                        nn_ = 2 * half + n
                        nt.matmul(pxw[:, 512 * n:512 * (n + 1)], xtc[kd],
                                  wih_sb[kd][:, 512 * nn_:512 * (nn_ + 1)],
                                  start=(kd == 0), stop=False)
                for n in range(2):
                    nn_ = 2 * half + n
                    nt.matmul(pxw[:, 512 * n:512 * (n + 1)], ones_sb[:],
                              biasrow_sb[:, 512 * nn_:512 * (nn_ + 1)],
                              start=False, stop=True)
                if half == 0:
                    ns.copy(xw[:, 0:1024], pxw[:])
                else:
                    nv.tensor_copy(xw[:, 1024:2048], pxw[:])
            pa1 = ps_a1.tile([128, 2], F32)
            for kd in range(2):
                nt.matmul(pa1[:], xtc[kd], u1t_sb[:, 2 * kd:2 * kd + 2],
                          start=(kd == 0), stop=(kd == 1))
            ns.copy(a1ch[:, r:r + 1], pa1[:, 0:1])
            return xw

        xw_tiles = {0: emit_xw_chunk(0)}

        for t in range(t_steps):
            r, t4 = divmod(t, 4)
            xw = xw_tiles[r]
            pz = ps_z.tile([BL, G], F32, tag="pz")
            # n-chunk outer: chunk n finishes early so gates can start sooner
            for n in range(4):
                nt.matmul(pz[:, 512 * n:512 * (n + 1)],
                          i32s_r[32 * t4:32 * (t4 + 1), :],
                          xw[32 * t4:32 * (t4 + 1), 512 * n:512 * (n + 1)],
                          start=True, stop=(t == 0),
                          tile_position=(32 * t4, 0))
                if t > 0:
                    for k in range(4):
                        nt.matmul(
                            pz[:, 512 * n:512 * (n + 1)],
                            hsT[:, (t - 1) * 128 + 32 * k:(t - 1) * 128 + 32 * (k + 1)],
                            whh_sb[k][:, 512 * n:512 * (n + 1)],
                            start=False, stop=(k == 3))
            # gates: z cols [i(0:512) f(512:1024) o(1024:1536) g(1536:2048)]
            sg = gate_pool.tile([BL, 1536], F32, tag="sg")
            ns.activation(sg[:], pz[:, 0:1536], AFT.Sigmoid)
            gg = gate_pool.tile([BL, 512], F32, tag="gg")
            ns.activation(gg[:], pz[:, 1536:2048], AFT.Tanh)
            tig = tmp_pool.tile([BL, H], F32, tag="tig")
            nv.tensor_tensor(tig[:], sg[:, 0:512], gg[:], op=ALU.mult)
            if t == 0:
                nv.tensor_copy(c_sb[:], tig[:])
            else:
                tfc = tmp_pool.tile([BL, H], F32, tag="tfc")
                nv.tensor_tensor(tfc[:], sg[:, 512:1024], c_sb[:], op=ALU.mult)
                nv.tensor_tensor(c_sb[:], tfc[:], tig[:], op=ALU.add)
            tcs = tmp_pool.tile([BL, H], F32, tag="tcs")
            ns.activation(tcs[:], c_sb[:], AFT.Tanh)
            h_t = h_pool.tile([BL, H], F32, tag="h")
            nv.tensor_tensor(h_t[:], sg[:, 1024:1536], tcs[:], op=ALU.mult)
            # a2[t] = h . u2  (per-partition dot)
            scr = scr_pool.tile([BL, H], F32, tag="scr")
            nv.scalar_tensor_tensor(scr[:], h_t[:], 1.0, u2b_sb[:],
                                    op0=ALU.bypass, op1=ALU.mult,
                                    accum_out=a2_sb[:, t:t + 1])
            # transpose h -> hsT[:, t*128:(t+1)*128] (converts to fp32r)
            phT = ps_hT.tile([128, 128], F32, tag="phT")
            for c in range(4):
                nt.transpose(phT[:, 32 * c:32 * (c + 1)],
                             h_t[:, 128 * c:128 * (c + 1)], i32s_f[0:32, :])
            ns.copy(hsT[:, t * 128:(t + 1) * 128], phT[:])
            # prefetch next xw chunk (3 steps of slack before it's consumed)
            if t4 == 0 and r + 1 < NRC:
                xw_tiles[r + 1] = emit_xw_chunk(r + 1)
                xw_tiles.pop(r - 1, None)

        loop_ctx.close()

        # ---- post-loop: attention scores + softmax + pooling ----
        post = ctx.enter_context(tc.tile_pool(name="post", bufs=1))
        ps_t = ctx.enter_context(tc.tile_pool(name="ps_t", bufs=2, space="PSUM"))
        ps_pool = ctx.enter_context(tc.tile_pool(name="ps_pool", bufs=4, space="PSUM"))
        stg_pool = ctx.enter_context(tc.tile_pool(name="stg", bufs=4))
        hsb_pool = ctx.enter_context(tc.tile_pool(name="hsb", bufs=2))

        # a1 assembly: a1bp[b, 4r+c] = a1ch[32c+b, r]
        a1bp = post.tile([BL, t_steps], F32, tag="a1bp")
        for c in range(4):
            nv.tensor_copy(a1bp[:].rearrange("b (r c) -> b r c", c=4)[:, :, c],
                           a1ch[32 * c:32 * (c + 1), :])
        abp = post.tile([BL, t_steps], F32, tag="abp")
        nv.tensor_tensor(abp[:], a1bp[:], a2_sb[:], op=ALU.add)
        ns.activation(abp[:], abp[:], AFT.Sigmoid)

        # softmax per slot k -> wT [t, 4b+k] (fp32r for the pooling matmul)
        wT = post.tile([t_steps, K * BL], F32R, tag="wT")
        for k in range(K):
            sc = post.tile([BL, t_steps], F32, tag=f"sc{k}")
            nv.tensor_tensor(sc[:], abp[:],
                             maskneg_sb[:, t_steps * k:t_steps * (k + 1)], op=ALU.add)
            mneg = post.tile([BL, 1], F32, tag=f"mneg{k}")
            nv.tensor_reduce(mneg[:], sc[:], axis=mybir.AxisListType.X,
                             op=ALU.max, negate=True)
            ek = post.tile([BL, t_steps], F32, tag=f"ek{k}")
            sk = post.tile([BL, 1], F32, tag=f"sk{k}")
            ns.activation(ek[:], sc[:], AFT.Exp, bias=mneg[:], accum_out=sk[:])
            rk = post.tile([BL, 1], F32, tag=f"rk{k}")
            nv.reciprocal(rk[:], sk[:])
            wk = post.tile([BL, t_steps], F32, tag=f"wk{k}")
            nv.tensor_scalar(out=wk[:], in0=ek[:], scalar1=rk[:],
                             scalar2=valid_sb[:, k:k + 1], op0=ALU.mult, op1=ALU.mult)
            # transpose into wT columns k::4  (wT[t, 4b+k])
            pwT = ps_t.tile([128, 32], F32, tag="pwT")
            nt.transpose(pwT[0:t_steps, :], wk[:], i32s_f[0:32, :])
            nv.tensor_copy(wT[:].rearrange("t (b k) -> t b k", k=4)[:, :, k],
                           pwT[0:t_steps, :])

        # pooling: per b, rebuild hs_b [t, h] via 4 PE transposes, then [4,T]@[T,H]
        hsT_r = hsT[:].rearrange("p (t c b) -> p t c b", c=4, b=BL)
        for b in range(BL):
            hsb = hsb_pool.tile([t_steps, H], F32R, tag="hsb")
            for c in range(4):
                pt = ps_t.tile([128, 128], F32R, tag="pt")
                nt.transpose(pt[0:t_steps, :], hsT_r[:, :, c, b], i128_r[:])
                if c % 2 == 0:
                    ns.copy(hsb[:, 128 * c:128 * (c + 1)], pt[0:t_steps, :])
                else:
                    nv.tensor_copy(hsb[:, 128 * c:128 * (c + 1)], pt[0:t_steps, :])
            pp = ps_pool.tile([K, H], F32, tag="pp")
            nt.matmul(pp[:], wT[0:t_steps, 4 * b:4 * (b + 1)], hsb[:],
                      start=True, stop=True)
            so = stg_pool.tile([K, H], F32, tag="so")
            ns.copy(so[:], pp[:])
            nc.sync.dma_start(d_out[K * b:K * (b + 1), :], so[:])

    nc.compile()
    return nc


def _host_prep(x, W_ih, W_hh, b_ih, b_hh, A1, A2, v1, lengths, label_len):
    assert int(label_len) == K
    perm = np.concatenate([np.arange(0, 512), np.arange(512, 1024),
                           np.arange(1536, 2048), np.arange(1024, 1536)])
    wih = np.ascontiguousarray(W_ih[perm].T, dtype=np.float32)          # [256, 2048]
    whh = np.ascontiguousarray(W_hh[perm].T, dtype=np.float32)          # [512, 2048]
    biasrow = ((b_ih + b_hh)[perm]).astype(np.float32).reshape(1, G)
    u1 = (v1 @ A1)[0].astype(np.float32)                                # [256]
    u2 = (v1 @ A2)[0].astype(np.float32)                                # [512]
    u1t = np.zeros((128, 4), dtype=np.float32)                          # [128, 4]
    u1t[:, 0] = u1[0:128]
    u1t[:, 2] = u1[128:256]
    u2b = np.ascontiguousarray(np.broadcast_to(u2, (BL, H)))            # [32, 512]
    i32s = np.zeros((128, 32), dtype=np.float32)
    i32s[np.arange(128), np.arange(128) % 32] = 1.0
    i128 = np.eye(128, dtype=np.float32)

    shared = dict(wih=wih, whh=whh, biasrow=biasrow, u1t=u1t, u2b=u2b,
                  i32s=i32s, i128=i128, onesrow=np.ones((1, 128), dtype=np.float32))

    in_maps = []
    for cidx in range(NC):
        sl = slice(cidx * BL, (cidx + 1) * BL)
        xc = x[:, sl, :]                                                # [T, 32, D]
        xT = np.ascontiguousarray(xc.reshape(T * BL, D).T, dtype=np.float32)
        ln = lengths[sl].astype(np.int64)
        t_start = np.maximum(ln - K, 0)
        t_k = t_start[:, None] + np.arange(K)[None, :]                  # [32, 4]
        valid = (t_k <= (ln[:, None] - 1))                              # [32, 4]
        tt = np.arange(T)
        mask = (tt[None, None, :] <= t_k[:, :, None]) & valid[:, :, None]  # [b, k, t]
        maskneg = np.where(mask, 0.0, NEG_INF).astype(np.float32)
        maskneg = np.ascontiguousarray(maskneg.reshape(BL, K * T))      # k-major cols
        in_maps.append(dict(shared, xT=xT, maskneg=maskneg,
                            valid=valid.astype(np.float32)))
    return in_maps


def kernel(**inputs) -> np.ndarray:
    inputs = {k: np.asarray(v) if not np.isscalar(v) else v for k, v in inputs.items()}
    in_maps = _host_prep(**inputs)
    if "nc" not in _cached:
        _cached["nc"] = _build_program()
    nc = _cached["nc"]
    res = run_bass_kernel_spmd(nc, in_maps, core_ids=list(range(NC)))
    outs = []
    for cidx in range(NC):
        o = res.results[cidx]["out"]                                    # [128, 512]
        outs.append(o.reshape(BL, K, H))
    return np.concatenate(outs, axis=0).astype(np.float32)              # [256, 4, 512]

